# revision 15
# baseline (speedup 1.0000x reference)
"""Trainium2 Bass kernel for nn_Attention (dense transformer attention block).

Reference computation (shapes fixed):
  x [2, 256, 48, 48] -> RMSNorm over channels -> 1x1 conv to qkv (8 heads, 64 dhead)
  -> prepend 4 learnable mem kv tokens -> softmax attention -> 1x1 conv out [2, 256, 48, 48]

Sharding: 8 cores = 2 batches x 4 head-pairs. Core c handles batch c//4 and
heads (2g, 2g+1), g = c%4. Each core computes its heads' attention and a
partial out-projection [256, 2304]; partials are ReduceScattered (chunked,
overlapped with compute) within each batch's 4-core group; each core returns
its 64-channel slice of the reduced output and the host reassembles.

Numerics: qkv projection in float32r (full-rate PE, ~19-bit); attention
matmuls (sim, attn@v, out-projection) in bf16 with fp32 psum accumulation.
Layout highlights:
  - x, xn in [channel, pos]; RMSNorm scale via all-ones-lhsT matmul that
    broadcasts the sum of squares to all 128 partitions.
  - q/k/v in [dhead(2 heads packed), pos]; sim matmuls row-packed (head A on
    PE rows 0-63, head B on 64-127, concurrent via row groups).
  - scores S^T [key, query] in psum; exp on ACT straight psum->sbuf; P @ v^T
    accumulated in psum with lhsT columns [ones | zeros*63 | v], giving the
    softmax denominator on partition 0 and out^T on partitions 64-127.
  - denominator: fast reciprocal (DVE) + partition broadcast (GpSimd).
"""
import numpy as np

import concourse.mybir as mybir
import concourse.tile as tile
from concourse import bacc
from concourse.bass_utils import run_bass_kernel_spmd
from concourse.masks import make_identity

F32 = mybir.dt.float32
F32R = mybir.dt.float32r
BF16 = mybir.dt.bfloat16
EXP = mybir.ActivationFunctionType.Exp
SQRT = mybir.ActivationFunctionType.Sqrt

DIM = 256
HEADS = 8
DHEAD = 64
MEM = 4
HID = 512
N = 48 * 48          # 2304 image positions
NK = N + MEM         # 2308 keys (mem tokens at the END: cols 2304:2308)
NJT = N // 128       # 18 image j-tiles
GROUPS = [[0, 1, 2, 3], [4, 5, 6, 7]]

# i-chunks of the query axis
CHUNKS = [(0, 512), (512, 512), (1024, 512), (1536, 512), (2048, 256)]
# reduce-scatter batches (chunk indices per collective, contiguous)
RS_BATCHES = [[0, 1], [2, 3], [4]]
RS_GROUP = {0: 0, 1: 0, 2: 1, 3: 1, 4: 2}


def build():
    nc = bacc.Bacc("TRN2", target_bir_lowering=False, debug=False,
                   enable_asserts=True, num_devices=8)
    x_d = nc.dram_tensor("x", [DIM, N], F32, kind="ExternalInput").ap()
    wqkv_d = nc.dram_tensor("wqkv", [DIM, 384], F32, kind="ExternalInput").ap()
    memk_d = nc.dram_tensor("memk", [128, MEM], F32, kind="ExternalInput").ap()
    memv_d = nc.dram_tensor("memv", [MEM, 2, DHEAD], F32, kind="ExternalInput").ap()
    woutT_d = nc.dram_tensor("woutT", [2, DHEAD, DIM], F32, kind="ExternalInput").ap()
    out_d = nc.dram_tensor("out", [DHEAD, N], F32, kind="ExternalOutput").ap()

    NCH = len(CHUNKS)
    with tile.TileContext(nc) as tc:
        with (
            tc.tile_pool(name="consts", bufs=1) as consts,
            tc.tile_pool(name="big", bufs=1) as big,
            tc.tile_pool(name="io", bufs=2) as io,
            tc.tile_pool(name="pP", bufs=3) as pP,
            tc.tile_pool(name="ps_s", bufs=2, space="PSUM") as ps_s,
            tc.tile_pool(name="ps_a", bufs=2, space="PSUM") as ps_a,
            tc.tile_pool(name="dram", bufs=1, space="DRAM") as dram,
        ):
            # ---------------- constants ----------------
            ident = consts.tile([128, 128], F32)
            make_identity(nc, ident)
            ones_f = consts.tile([128, 1], F32)
            nc.vector.memset(ones_f[:, :], 1.0)
            zeros_f = consts.tile([128, 1], F32)
            nc.vector.memset(zeros_f[:, :], 0.0)
            ones_r = consts.tile([128, 128], F32R)
            nc.vector.tensor_copy(ones_r[:, :], ones_f[:, :].to_broadcast((128, 128)))

            # ---------------- collective warmup ----------------
            # the first collective on a NEFF pays ~60us of firmware cold
            # start; absorb it behind the compute phase with a tiny dummy.
            warm_sb = consts.tile([1, 32], F32)
            nc.vector.memset(warm_sb[:, :], 0.0)
            wi = dram.tile([1, 32], F32, tag="wi")
            wo = dram.tile([1, 32], F32, tag="wo")
            nc.sync.dma_start(out=wi[:, :], in_=warm_sb[:, :])
            nc.gpsimd.collective_compute(
                "AllReduce", mybir.AluOpType.add,
                replica_groups=GROUPS,
                ins=[wi[:, :].opt()],
                outs=[wo[:, :].opt()],
            )

            # ---------------- vT tile skeletons (no data deps) ----------------
            # per (head, jt): [key(128 part), 128]: col 0 ones, 1:64 zeros,
            # 64:128 v^T.  jt == NJT holds the 4 mem tokens on rows 0:4.
            vT = [[None, None] for _ in range(NJT + 1)]
            for jt in range(NJT + 1):
                for h in range(2):
                    t = big.tile([128, 128], BF16, tag=f"vT{h}_{jt}")
                    vT[jt][h] = t
                    nc.vector.tensor_copy(
                        t[:, 0:1], ones_f[:, :].to_broadcast((128, 1)))
                    nc.vector.tensor_copy(
                        t[:, 1:64], zeros_f[:, :].to_broadcast((128, 63)))

            # ---------------- load inputs ----------------
            xs = [[None] * NCH, [None] * NCH]
            dma_engines = [nc.sync, nc.scalar, nc.gpsimd]
            for ci, (c0, cw) in enumerate(CHUNKS):
                for kt in range(2):
                    t = big.tile([128, cw], F32, tag=f"x{kt}_{ci}")
                    xs[kt][ci] = t
                    eng = dma_engines[(2 * ci + kt) % 3]
                    eng.dma_start(
                        out=t[:, :], in_=x_d[128 * kt:128 * kt + 128, c0:c0 + cw])

            wq_f = io.tile([128, 2, 384], F32)
            nc.sync.dma_start(out=wq_f[:, 0, :], in_=wqkv_d[0:128, :])
            nc.sync.dma_start(out=wq_f[:, 1, :], in_=wqkv_d[128:256, :])
            wq = consts.tile([128, 2, 384], F32R)
            nc.vector.tensor_copy(wq[:, :, :], wq_f[:, :, :])

            memk_f = io.tile([128, MEM], F32)
            nc.sync.dma_start(out=memk_f[:, :], in_=memk_d)
            kmem = consts.tile([128, MEM], BF16)
            nc.vector.tensor_copy(kmem[:, :], memk_f[:, :])
            memv_f = io.tile([MEM, 2, DHEAD], F32)
            nc.sync.dma_start(out=memv_f[:, :, :], in_=memv_d)
            for h in range(2):
                nc.vector.tensor_copy(vT[NJT][h][0:MEM, 64:128], memv_f[:, h, :])

            # wout lhsT tiles, one per head, data on partitions 64..127
            woutA_f = io.tile([128, DIM], F32, tag="woutA_f")
            woutB_f = io.tile([128, DIM], F32, tag="woutB_f")
            nc.sync.dma_start(out=woutA_f[64:128, :], in_=woutT_d[0, :, :])
            nc.sync.dma_start(out=woutB_f[64:128, :], in_=woutT_d[1, :, :])
            woutA = consts.tile([128, DIM], BF16, tag="woutA")
            woutB = consts.tile([128, DIM], BF16, tag="woutB")
            nc.vector.tensor_copy(woutA[64:128, :], woutA_f[64:128, :])
            nc.vector.tensor_copy(woutB[64:128, :], woutB_f[64:128, :])
            wouts = [woutA, woutB]

            # ------------- per-chunk prep / attention, zigzag-interleaved ----
            SQUARE = mybir.ActivationFunctionType.Square
            qs, ks, vs = [None] * NCH, [None] * NCH, [None] * NCH

            def prep_chunk(ci):
                c0, cw = CHUNKS[ci]
                xsq0 = pP.tile([128, 512], F32R, tag="xsq0", name=f"xsq0_{ci}")
                xsq1 = pP.tile([128, 512], F32R, tag="xsq1", name=f"xsq1_{ci}")
                nc.scalar.activation(xsq0[:, 0:cw], xs[0][ci][:, :], SQUARE)
                nc.scalar.activation(xsq1[:, 0:cw], xs[1][ci][:, :], SQUARE)
                sb_ps = ps_a.tile([128, 512], F32, tag="a0", name=f"ssq_{ci}")
                nc.tensor.matmul(sb_ps[:, 0:cw], ones_r[:, :],
                                 xsq0[:, 0:cw], start=True, stop=False)
                nc.tensor.matmul(sb_ps[:, 0:cw], ones_r[:, :],
                                 xsq1[:, 0:cw], start=False, stop=True)
                sinv = pP.tile([128, 512], F32, tag="sinv", name=f"sinv_{ci}")
                nc.scalar.activation(sinv[:, 0:cw], sb_ps[:, 0:cw], SQRT,
                                     scale=1.0 / 256.0)
                nc.vector.reciprocal_approx_fast(sinv[:, 0:cw], sinv[:, 0:cw])
                xr0 = pP.tile([128, 512], F32R, tag="xn0", name=f"xr0_{ci}")
                xr1 = pP.tile([128, 512], F32R, tag="xn1", name=f"xr1_{ci}")
                nc.vector.tensor_copy(xr0[:, 0:cw], xs[0][ci][:, :])
                nc.vector.tensor_copy(xr1[:, 0:cw], xs[1][ci][:, :])
                xrs = [xr0, xr1]

                qc = big.tile([128, cw], BF16, tag=f"q{ci}", name=f"q_{ci}")
                kc = big.tile([128, cw], BF16, tag=f"k{ci}", name=f"k_{ci}")
                vc = big.tile([128, cw], F32, tag=f"v{ci}", name=f"v_{ci}")
                qs[ci], ks[ci], vs[ci] = qc, kc, vc
                for m, dst in ((1, kc), (2, vc), (0, qc)):
                    qp = ps_a.tile([128, 512], F32, tag="a0", name=f"qkv_{ci}_{m}")
                    for kt in range(2):
                        nc.tensor.matmul(
                            qp[:, 0:cw],
                            wq[:, kt, m * 128:(m + 1) * 128],
                            xrs[kt][:, 0:cw],
                            start=(kt == 0), stop=(kt == 1),
                        )
                    nc.vector.tensor_mul(dst[:, :], qp[:, 0:cw], sinv[:, 0:cw])
                # transposes for this chunk's j-tiles
                for jl in range(cw // 128):
                    jt = c0 // 128 + jl
                    for h in range(2):
                        tp = ps_a.tile([128, 512], F32, tag="a1",
                                       name=f"tp_{jt}_{h}")
                        nc.tensor.transpose(
                            tp[0:128, 0:64],
                            vc[64 * h:64 * h + 64, jl * 128:(jl + 1) * 128],
                            ident[64 * h:64 * h + 64, 64 * h:64 * h + 64],
                        )
                        nc.vector.tensor_copy(vT[jt][h][:, 64:128], tp[:, 0:64])

            accs_by_ci = [None] * NCH

            def attn_part(ci, jts):
                c0, cw = CHUNKS[ci]
                if accs_by_ci[ci] is None:
                    acc0 = ps_a.tile([128, 512], F32, tag="a0", name=f"acc0_{ci}")
                    acc1 = ps_a.tile([128, 512], F32, tag="a1", name=f"acc1_{ci}")
                    accs_by_ci[ci] = [acc0, acc1]
                accs = accs_by_ci[ci]
                for jt in jts:
                    s_ps = ps_s.tile([128, 2, 512], F32, tag="s",
                                     name=f"s_{ci}_{jt}")
                    if jt < NJT:
                        km = 128
                        klhs = [ks[jt // 4][64 * h:64 * h + 64,
                                            (jt % 4) * 128:(jt % 4) * 128 + 128]
                                for h in range(2)]
                    else:
                        km = MEM
                        klhs = [kmem[64 * h:64 * h + 64, :] for h in range(2)]
                    for h in range(2):
                        nc.tensor.matmul(
                            s_ps[0:km, h, 0:cw],
                            klhs[h],
                            qs[ci][64 * h:64 * h + 64, :],
                            start=True, stop=True,
                        )
                    P = pP.tile([128, 2, 512], BF16, tag="P", name=f"P_{ci}_{jt}")
                    nc.scalar.activation(P[0:km, :, 0:cw], s_ps[0:km, :, 0:cw], EXP)
                    for h in range(2):
                        nc.tensor.matmul(
                            accs[h][:, 0:cw],
                            vT[jt][h][0:km, :],
                            P[0:km, h, 0:cw],
                            start=(jt == 0), stop=(jt == NJT),
                            skip_group_check=True,
                        )

            def finish_chunk(ci):
                c0, cw = CHUNKS[ci]
                accs = accs_by_ci[ci]
                rb = pP.tile([128, 2, 512], F32, tag="rb", name=f"rb_{ci}")
                for h in range(2):
                    nc.vector.reciprocal_approx_fast(
                        rec[0:1, h, 0:cw], accs[h][0:1, 0:cw])
                    nc.gpsimd.partition_broadcast(rb[:, h, 0:cw], rec[0:1, h, 0:cw])
                oT0 = pP.tile([128, 512], BF16, tag="oT0", name=f"oT0_{ci}")
                oT1 = pP.tile([128, 512], BF16, tag="oT1", name=f"oT1_{ci}")
                oTs = [oT0, oT1]
                for h in range(2):
                    nc.vector.tensor_mul(
                        oTs[h][64:128, 0:cw], accs[h][64:128, 0:cw],
                        rb[64:128, h, 0:cw])
                osb = pP.tile([128, 2, 512], F32, tag="osb", name=f"osb_{ci}")
                for mt in range(2):
                    op = ps_a.tile([128, 512], F32, tag=f"a{mt}", name=f"op_{ci}_{mt}")
                    for h in range(2):
                        nc.tensor.matmul(
                            op[:, 0:cw],
                            wouts[h][64:128, mt * 128:(mt + 1) * 128],
                            oTs[h][64:128, 0:cw],
                            start=(h == 0), stop=(h == 1),
                        )
                    nc.vector.tensor_copy(osb[:, mt, 0:cw], op[:, 0:cw])
                grp = RS_GROUP[ci]
                g0 = CHUNKS[RS_BATCHES[grp][0]][0]
                bi = bis[grp]
                nc.sync.dma_start(out=bi[0, :, c0 - g0:c0 - g0 + cw],
                                  in_=osb[:, 0, 0:cw])
                nc.sync.dma_start(out=bi[1, :, c0 - g0:c0 - g0 + cw],
                                  in_=osb[:, 1, 0:cw])
                if ci == RS_BATCHES[grp][-1]:
                    nc.gpsimd.collective_compute(
                        "ReduceScatter", mybir.AluOpType.add,
                        replica_groups=GROUPS,
                        ins=[bi[:, :, :].opt()],
                        outs=[bos[grp][:, :].opt()],
                    )

            bis, bos = [], []
            for grp, cis in enumerate(RS_BATCHES):
                gw = sum(CHUNKS[ci][1] for ci in cis)
                bis.append(dram.tile([2, 128, gw], F32, tag=f"bi{grp}",
                                     name=f"bi_{grp}"))
                bos.append(dram.tile([DHEAD, gw], F32, tag=f"bo{grp}",
                                     name=f"bo_{grp}"))
            rec = io.tile([1, 2, 512], F32, tag="rec")

            for ci in range(NCH):
                prep_chunk(ci)
            for ci in range(NCH):
                lo = 0 if ci == 0 else 2
                attn_part(ci, range(lo, NJT + 1))
                if ci + 1 < NCH:
                    attn_part(ci + 1, range(0, 2))
                finish_chunk(ci)
            for grp, cis in enumerate(RS_BATCHES):
                g0 = CHUNKS[cis[0]][0]
                gw = sum(CHUNKS[ci][1] for ci in cis)
                nc.sync.dma_start(out=out_d[:, g0:g0 + gw], in_=bos[grp][:, :])
    nc.compile()
    return nc


_NC = None
_last_in_maps = None


def _get_nc():
    global _NC
    if _NC is None:
        _NC = build()
    return _NC


def make_in_maps(x, gamma, mem_kv, w_qkv, w_out):
    x = np.asarray(x, np.float32)
    gamma = np.asarray(gamma, np.float32).reshape(DIM)
    mem_kv = np.asarray(mem_kv, np.float32)
    w_qkv = np.asarray(w_qkv, np.float32)
    w_out = np.asarray(w_out, np.float32)

    g1 = 1.0 + gamma  # [256]
    scale = DHEAD ** -0.5
    in_maps = []
    for core in range(8):
        b, g = core // 4, core % 4
        hA, hB = 2 * g, 2 * g + 1
        blocks = []
        for t in range(3):  # q, k, v
            for h in (hA, hB):
                wblk = w_qkv[t * HID + h * DHEAD: t * HID + (h + 1) * DHEAD, :]
                if t == 0:
                    wblk = wblk * scale
                blocks.append(wblk.T)  # [256, 64]
        wqkvT = np.concatenate(blocks, axis=1) * g1[:, None]  # [256, 384]
        memk = np.concatenate(
            [mem_kv[0, hA].T, mem_kv[0, hB].T], axis=0)  # [128, 4]
        memv = np.stack([mem_kv[1, hA], mem_kv[1, hB]], axis=1)  # [4, 2, 64]
        woutT = np.stack(
            [w_out[:, hA * DHEAD:(hA + 1) * DHEAD].T,
             w_out[:, hB * DHEAD:(hB + 1) * DHEAD].T], axis=0)  # [2, 64, 256]
        in_maps.append({
            "x": np.ascontiguousarray(x[b].reshape(DIM, N)),
            "wqkv": np.ascontiguousarray(wqkvT),
            "memk": np.ascontiguousarray(memk),
            "memv": np.ascontiguousarray(memv),
            "woutT": np.ascontiguousarray(woutT),
        })
    return in_maps


def kernel(x, gamma, mem_kv, w_qkv, w_out):
    global _last_in_maps
    in_maps = make_in_maps(x, gamma, mem_kv, w_qkv, w_out)
    _last_in_maps = in_maps
    nc = _get_nc()
    res = run_bass_kernel_spmd(nc, in_maps, core_ids=list(range(8)))
    out = np.empty((2, DIM, N), np.float32)
    for core in range(8):
        b, g = core // 4, core % 4
        out[b, 64 * g:64 * g + 64, :] = res.results[core]["out"]
    return out.reshape(2, DIM, 48, 48)
